# revision 1
# baseline (speedup 1.0000x reference)
"""Trainium2 kernel for nn_MemoryAttentionLayer (retrieval_knn).

Strategy (sharding_hint: shard memory rows across 8 cores, replicate queries):

Device L1 (8 cores, SPMD — 99.8% of FLOPs):
  - each core holds a row-shard of memory_keys, pre-transposed to
    keysT [kd=128 part, slots] bf16 so the PE can stream it as the moving
    operand against stationary qT [kd, 128 queries].
  - PE: scores[q, slot] in PSUM, fp32 accum, 512-slot tiles.
  - drain is split between the only two engines that can read PSUM:
      * DVE tiles: exact reduce_max over vpr=64 -> rowmax[q, row] (8 rows/tile)
      * ACT tiles: activation(Relu, bias=-t_q, accum_out) -> hinge[q, tile] =
        sum(relu(s - t_q)); hinge > 0  <=>  tile contains a slot above t_q.
Host (0.2% of FLOPs):
  - queries projection (tiny), per-query threshold t_q estimate,
  - flag candidate rows (DVE: rowmax >= t_q - margin; ACT: hinge > 0),
  - exact fp32 rescore of the ~70 flagged rows/query -> exact top-32 and
    within-row argmax (count-check fallback makes this sound for any data),
  - exact attention tail: softmax over 32, retrieved, update matmul,
    scatter-add, layer norm.
"""

import numpy as np
import ml_dtypes

bf16 = ml_dtypes.bfloat16

# ---- problem constants (hardcoded per spec) ----
N_CORES = 8
B, T, H = 4, 512, 768
NM = 128                      # n_mentions / queries
ROWS, VPR, KD = 16384, 64, 128
K_TOP = 32
LN_EPS = 1e-12

NSLOTS = ROWS * VPR           # 1048576
SPC = NSLOTS // N_CORES       # 131072 slots per core
RPC = ROWS // N_CORES         # 2048 rows per core
TILE = 512                    # slots per PSUM tile / matmul
NTILES = SPC // TILE          # 256 tiles per core
RPT = TILE // VPR             # 8 rows per tile

# ---- tunables ----
import os
CHUNK = int(os.environ.get("MK_CHUNK", 8192))   # slots per DMA chunk
N_ACT_OF_16 = int(os.environ.get("MK_NACT", 8)) # ACT-hinge tiles per 16
HINGE_SBUF = os.environ.get("MK_HSBUF", "1") == "1"
ACT_SPREAD = os.environ.get("MK_SPREAD", "1") == "1"  # interleave ACT/DVE tiles


def _act_positions(tpc, n_act_chunk, spread):
    """Which of the tpc tile positions in a chunk drain via ACT."""
    if not spread or n_act_chunk == 0:
        return set(range(n_act_chunk))
    # Bresenham spread: evenly distribute n_act among tpc positions
    return {mi for mi in range(tpc) if (mi * n_act_chunk) % tpc < n_act_chunk}
Z_THRESH = 3.70               # t_q = z * sigma_q
MARGIN = 0.03                 # bf16 score error allowance for flagging

_NC_CACHE: dict = {}


def _build_nc(spc=SPC, chunk=CHUNK, n_act_of_16=None, hinge_sbuf=None,
              spread=None):
    import concourse.bacc as bacc
    import concourse.mybir as mybir
    from concourse import tile

    if n_act_of_16 is None:
        n_act_of_16 = N_ACT_OF_16
    if hinge_sbuf is None:
        hinge_sbuf = HINGE_SBUF
    if spread is None:
        spread = ACT_SPREAD

    ntiles = spc // TILE
    nchunks = spc // chunk
    tiles_per_chunk = chunk // TILE
    # per-chunk split: first n_act_chunk tiles drain via ACT hinge, rest DVE
    n_act_chunk = tiles_per_chunk * n_act_of_16 // 16
    n_dve_chunk = tiles_per_chunk - n_act_chunk
    act_pos = _act_positions(tiles_per_chunk, n_act_chunk, spread)

    # Bacc (not raw Bass): its compile() legalizes multi-wait instructions
    # (move_matmul_waits_to_ldweights + generate_event_semaphores) — walrus
    # caps compute instructions at ONE sync wait.
    nc = bacc.Bacc()
    keysT_d = nc.dram_tensor("keysT", [KD, spc], mybir.dt.bfloat16,
                             kind="ExternalInput")
    qT_d = nc.dram_tensor("qT", [KD, NM], mybir.dt.bfloat16,
                          kind="ExternalInput")
    tqneg_d = nc.dram_tensor("tqneg", [NM, 1], mybir.dt.float32,
                             kind="ExternalInput")
    # packed outputs: only the regions actually written on-chip
    rowmax_d = nc.dram_tensor("rowmax", [NM, nchunks, max(n_dve_chunk, 1) * RPT],
                              mybir.dt.float32, kind="ExternalOutput")
    hinge_d = nc.dram_tensor("hinge", [NM, nchunks, max(n_act_chunk, 1)],
                             mybir.dt.float32, kind="ExternalOutput")

    with tile.TileContext(nc) as tc:
        with (
            tc.tile_pool(name="kpool", bufs=3) as kpool,
            tc.tile_pool(name="const", bufs=1) as const_pool,
            tc.tile_pool(name="outs", bufs=1) as out_pool,
            tc.tile_pool(name="scr", bufs=4) as scr_pool,
            tc.tile_pool(name="ps", bufs=8, space="PSUM") as ps_pool,
        ):
            q_t = const_pool.tile([KD, NM], mybir.dt.bfloat16)
            nc.sync.dma_start(q_t[:], qT_d[:])
            tq_t = const_pool.tile([NM, 1], mybir.dt.float32)
            nc.sync.dma_start(tq_t[:], tqneg_d[:])

            rm_t = out_pool.tile([NM, nchunks * max(n_dve_chunk, 1) * RPT],
                                 mybir.dt.float32)
            hg_t = out_pool.tile([NM, nchunks * max(n_act_chunk, 1)],
                                 mybir.dt.float32)

            # consume the tq DMA dep on a throwaway ACT op up front
            tq_warm = const_pool.tile([NM, 1], mybir.dt.float32)
            nc.scalar.copy(tq_warm[:], tq_t[:])
            if n_act_chunk == 0:
                nc.scalar.memzero(hg_t[:])
            if n_dve_chunk == 0:
                nc.scalar.memzero(rm_t[:])

            # packed column index per global tile (chunk-major, rank-ordered)
            hg_cols = {}
            rm_cols = {}
            for _ci in range(nchunks):
                _a = _d = 0
                for _mi in range(tiles_per_chunk):
                    _ti = _ci * tiles_per_chunk + _mi
                    if _mi in act_pos:
                        hg_cols[_ti] = _ci * n_act_chunk + _a
                        _a += 1
                    else:
                        rm_cols[_ti] = _ci * n_dve_chunk + _d
                        _d += 1

            for ci in range(nchunks):
                k_t = kpool.tile([KD, chunk], mybir.dt.bfloat16)
                nc.sync.dma_start(k_t[:], keysT_d[:, ci * chunk:(ci + 1) * chunk])
                for mi in range(tiles_per_chunk):
                    ti = ci * tiles_per_chunk + mi
                    ps = ps_pool.tile([NM, TILE], mybir.dt.float32)
                    nc.tensor.matmul(ps[:], q_t[:],
                                     k_t[:, mi * TILE:(mi + 1) * TILE],
                                     start=True, stop=True)
                    if mi in act_pos:
                        if hinge_sbuf:
                            scr = scr_pool.tile([NM, TILE], mybir.dt.bfloat16)
                            out_ap = scr[:]
                        else:
                            out_ap = ps[:]
                        nc.scalar.activation(
                            out_ap, ps[:], mybir.ActivationFunctionType.Relu,
                            bias=tq_t[:, 0:1], scale=1.0,
                            accum_out=hg_t[:, hg_cols[ti]:hg_cols[ti] + 1])
                    else:
                        o0 = rm_cols[ti] * RPT
                        nc.vector.reduce_max(
                            rm_t[:, o0:o0 + RPT],
                            ps[:].rearrange("p (r v) -> p r v", v=VPR),
                            axis=mybir.AxisListType.X)

            nc.sync.dma_start(
                rowmax_d[:],
                rm_t[:, :nchunks * max(n_dve_chunk, 1) * RPT].rearrange(
                    "p (c t) -> p c t", c=nchunks))
            nc.sync.dma_start(
                hinge_d[:],
                hg_t[:, :nchunks * max(n_act_chunk, 1)].rearrange(
                    "p (c t) -> p c t", c=nchunks))
    nc.finalize()   # Bacc: runs compile() — reg alloc + multi-wait legalization
    return nc


def _get_nc():
    key = (SPC, CHUNK, N_ACT_OF_16, HINGE_SBUF)
    if key not in _NC_CACHE:
        _NC_CACHE[key] = _build_nc()
    return _NC_CACHE[key]


# ---------------- host side ----------------

def _host_queries(enc2d, mbp, msp, mep, qw, qb):
    start_enc = enc2d[mbp * T + msp]
    end_enc = enc2d[mbp * T + mep]
    q = np.concatenate([start_enc, end_enc], -1).astype(np.float32) @ qw + qb
    return q.astype(np.float32)


def _estimate_tq(queries, mem_keys):
    # deterministic spread sample of 256 rows -> per-query score sigma
    samp_rows = np.arange(0, ROWS, ROWS // 256)[:256]
    samp = mem_keys[samp_rows].reshape(-1, KD)          # [16384, KD]
    s = queries @ samp.T.astype(np.float32)
    sigma = s.std(axis=1) + 1e-12
    return (Z_THRESH * sigma).astype(np.float32)


def _selection(queries, mem_keys, t_q, rowmax_all, hinge_all):
    """Exact top-32 rows + within-row argmax per query, from approximate
    device stats plus exact host rescore of flagged rows."""
    tpc = CHUNK // TILE
    n_act_chunk = tpc * N_ACT_OF_16 // 16
    act_pos = _act_positions(tpc, n_act_chunk, ACT_SPREAD)
    act_tile = np.array([(ti % tpc) in act_pos for ti in range(NTILES)])
    act_group = np.tile(act_tile, N_CORES)              # [ROWS // RPT]
    dve_row = np.repeat(~act_group, RPT)                # [ROWS]

    flags = np.zeros((NM, ROWS), bool)
    flags[:, dve_row] = rowmax_all[:, dve_row] >= (t_q[:, None] - MARGIN)
    hinge_pos = np.nan_to_num(hinge_all, nan=1.0, posinf=1.0) > 0
    flags |= np.repeat(hinge_pos & act_group[None, :], RPT, axis=1)

    keys2d = mem_keys.reshape(NSLOTS, KD)
    top_ids = np.empty((NM, K_TOP), np.int64)
    top_vals = np.empty((NM, K_TOP), np.float32)
    n_flagged = 0
    n_fallback = 0
    for q in range(NM):
        cand = np.nonzero(flags[q])[0]
        n_flagged += cand.size
        kc = mem_keys[cand]                              # [n, VPR, KD]
        s = np.einsum('d,nvd->nv', queries[q], kc.astype(np.float32),
                      optimize=True)
        vals = s.max(-1)
        if cand.size < K_TOP or (vals >= t_q[q]).sum() < K_TOP:
            # threshold estimate was too aggressive -> exact full rescore
            n_fallback += 1
            s = (queries[q] @ keys2d.T.astype(np.float32)).reshape(ROWS, VPR)
            cand = np.arange(ROWS)
            vals = s.max(-1)
        wi = s.argmax(-1)
        order = np.argsort(-vals, kind='stable')[:K_TOP]
        top_ids[q] = cand[order] * VPR + wi[order]
        top_vals[q] = vals[order]
    stats = dict(flagged_rows_per_q=n_flagged / NM, fallback_queries=n_fallback)
    return top_ids, stats


def _tail(enc2d, mbp, msp, mask, mem_keys, queries, top_ids, uw, ub, g, bb):
    keys2d = mem_keys.reshape(NSLOTS, KD)
    top_keys = keys2d[top_ids]                           # [NM, K, KD]
    s = np.einsum('qd,qkd->qk', queries, top_keys).astype(np.float32)
    s = s - s.max(-1, keepdims=True)
    e = np.exp(s)
    attn = e / e.sum(-1, keepdims=True)
    retrieved = np.einsum('qk,qkd->qd', attn, top_keys).astype(np.float32)
    retrieved *= mask[:, None]
    update = retrieved @ uw + ub
    upd = enc2d.copy()
    np.add.at(upd, mbp * T + msp, update)
    mu = upd.mean(-1, keepdims=True)
    var = ((upd - mu) ** 2).mean(-1, keepdims=True)
    out = (upd - mu) / np.sqrt(var + LN_EPS) * g + bb
    return out.astype(np.float32).reshape(B, T, H)


def _prep_in_maps(mem_keys, queries, t_q):
    keys2d_b = np.ascontiguousarray(mem_keys.reshape(NSLOTS, KD)).astype(bf16)
    qT = np.ascontiguousarray(queries.T).astype(bf16)
    tqneg = (-t_q)[:, None].astype(np.float32)
    in_maps = []
    for c in range(N_CORES):
        shard = np.ascontiguousarray(keys2d_b[c * SPC:(c + 1) * SPC].T)
        in_maps.append({"keysT": shard, "qT": qT, "tqneg": tqneg})
    return in_maps


def run_full(inputs, trace=False, trace_cores=None):
    from concourse.bass_utils import run_bass_kernel_spmd

    enc = np.asarray(inputs['encoded_input'], np.float32)
    mbp = np.asarray(inputs['mention_batch_positions']).astype(np.int64)
    msp = np.asarray(inputs['mention_start_positions']).astype(np.int64)
    mep = np.asarray(inputs['mention_end_positions']).astype(np.int64)
    mask = np.asarray(inputs['mention_mask'], np.float32)
    mem_keys = np.asarray(inputs['memory_keys'], np.float32)
    qw = np.asarray(inputs['query_w'], np.float32)
    qb = np.asarray(inputs['query_b'], np.float32)
    uw = np.asarray(inputs['update_w'], np.float32)
    ub = np.asarray(inputs['update_b'], np.float32)
    g = np.asarray(inputs['ln_gamma'], np.float32)
    bb = np.asarray(inputs['ln_beta'], np.float32)

    enc2d = enc.reshape(B * T, H)
    queries = _host_queries(enc2d, mbp, msp, mep, qw, qb)
    t_q = _estimate_tq(queries, mem_keys)
    in_maps = _prep_in_maps(mem_keys, queries, t_q)

    nc = _get_nc()
    res = run_bass_kernel_spmd(nc, in_maps, list(range(N_CORES)),
                               trace=trace, trace_cores=trace_cores)

    # unpack packed per-core outputs into full-size [NM, ROWS] / [NM, groups]
    tpc = CHUNK // TILE
    n_act_chunk = tpc * N_ACT_OF_16 // 16
    n_dve_chunk = tpc - n_act_chunk
    nchunks = SPC // CHUNK
    act_pos = sorted(_act_positions(tpc, n_act_chunk, ACT_SPREAD))
    dve_pos = [mi for mi in range(tpc) if mi not in act_pos]
    rowmax_all = np.full((NM, ROWS), -np.inf, np.float32)
    hinge_all = np.zeros((NM, ROWS // RPT), np.float32)
    for c in range(N_CORES):
        if n_dve_chunk > 0:
            rm = res.results[c]["rowmax"].reshape(NM, nchunks, n_dve_chunk, RPT)
            grid = np.full((NM, nchunks, tpc, RPT), -np.inf, np.float32)
            grid[:, :, dve_pos, :] = rm
            rowmax_all[:, c * RPC:(c + 1) * RPC] = grid.reshape(NM, RPC)
        if n_act_chunk > 0:
            hg = res.results[c]["hinge"].reshape(NM, nchunks, n_act_chunk)
            hgrid = np.zeros((NM, nchunks, tpc), np.float32)
            hgrid[:, :, act_pos] = hg
            hinge_all[:, c * NTILES:(c + 1) * NTILES] = hgrid.reshape(NM, NTILES)

    top_ids, stats = _selection(queries, mem_keys, t_q, rowmax_all, hinge_all)
    out = _tail(enc2d, mbp, msp, mask, mem_keys, queries, top_ids, uw, ub, g, bb)
    return out, res, stats


def kernel(**inputs) -> np.ndarray:
    out, _, _ = run_full(inputs, trace=False)
    return out

